# revision 1
# baseline (speedup 1.0000x reference)
"""FLAMETex kernel for Trainium2 (8 NeuronCores, Bass/Tile).

Reference computes tex = mean + basis @ texcode^T over the FULL 786432-row
texture, then downsamples 2x, flips channels (BGR), and gathers 5023 UV
points.  Only 3*5023 = 15069 texture rows can ever reach the output, and
the row indices depend only on uv_coords (an input).  So: compute the
gather indices on the host, gather the needed basis/mean rows, and run a
small (15104 x 201) @ (201 x 8) GEMM on device, row-sharded over the 8
cores (1888 rows each: 14 m-tiles of 128 + one of 96).

Per-core device layout: blob (201, 1896) f32 in DRAM; cols 0:8 hold
[texcode | ones]^T (mean folded in as the 201st contraction row), cols 8:
hold the gathered [basis | mean]^T shard.  The GEMM runs with the basis
slices as the STATIONARY operand (M<=128, full PE array) and the 8-column
x operand MOVING: per m-tile, the two contraction chunks (rows 0:128 /
73 rows 128:201) run as a back-to-back start/stop accumulation pair into
the tile's 8-column slice of a single (128, 120) PSUM bank -- at most one
open accumulation group per bank, which hardware requires (group state is
bank-granular; a two-pass all-c0-then-all-c1 order returns wrong data).
One DVE copy drains the bank; one DMA writes out_c (128, 120) = R-shard
in (tile, row)-interleaved layout that the host untangles.

Perf structure (TimelineSim-guided, 28.4us -> 10.9us/core):
 - chunk-0 column pieces (512,512,512,352) stream on the sync-engine
   HWDGE; chunk-1 goes through gpsimd/SWDGE in 4 pieces so the two DGE
   paths run in parallel;
 - five tiny "hold" matmuls in front wait on the first DMA and fill the
   PE sequencer's run-ahead window, so every real matmul is costed after
   ~3.3us (full p-state tier); on hardware they are 27ns each;
 - fp32 throughout (fp32r measured at ~1.5e-4 rel err - too coarse for
   an fp32-envelope gate; fp32 gives ~7e-8).
"""

import hashlib
import os
import shutil

import numpy as np

import concourse.bacc as bacc
import concourse.bass2jax as bass2jax
import concourse.mybir as mybir
import concourse.tile as tile
from concourse.bass_utils import run_bass_kernel_spmd

B = 8
K = 200
N_UV = 5023
V = 786432
ROWS = 3 * N_UV          # 15069 gathered texture rows
N_CORES = 8
PER_CORE = 1888          # 14 m-tiles of 128 + one of 96; 8 * 1888 = 15104 >= 15069
ROWS_PAD = N_CORES * PER_CORE
KA = K + 1               # contraction with the mean folded in
KC = 128                 # first contraction chunk (partition dim)
KC1 = KA - KC            # 73 rows in the second chunk
AW = B + PER_CORE        # blob width
MT = 128                 # m-tile height (PSUM partitions)
MT_HEIGHTS = (MT,) * 14 + (96,)
NMT = len(MT_HEIGHTS)    # 15
C0_GROUPS = (512, 512, 512, 352)
N_C1 = 4
N_HOLD = 5

_NC_CACHE = {}
_NEFF_CACHE_ROOT = "/tmp/bass_neff_cache"


def _install_neff_cache():
    """Cache compiled NEFFs by BIR content hash across processes.

    The bass2jax neuronx_cc_hook recompiles the identical BIR (a multi-
    minute walrus run with birsim enabled) on every fresh process. The
    kernel's BIR serialization is deterministic, so a sha256-keyed copy of
    the NEFF makes repeat cold starts ~2s instead of minutes. Falls back
    to the original compile on any cache error.
    """
    if getattr(bass2jax, "_flametex_neff_cache", False):
        return
    orig = getattr(bass2jax, "compile_bir_kernel", None)
    if orig is None:
        return

    def cached(bir_json, tmpdir, neff_name="file.neff"):
        key = hashlib.sha256(bir_json).hexdigest()
        cpath = os.path.join(_NEFF_CACHE_ROOT, key, "file.neff")
        dst = os.path.join(tmpdir, neff_name)
        try:
            if os.path.exists(cpath):
                shutil.copy(cpath, dst)
                return dst
        except OSError:
            pass
        neff = orig(bir_json, tmpdir, neff_name=neff_name)
        try:
            os.makedirs(os.path.dirname(cpath), exist_ok=True)
            tmp = cpath + f".tmp{os.getpid()}"
            shutil.copy(neff, tmp)
            os.replace(tmp, cpath)
        except OSError:
            pass
        return neff

    bass2jax.compile_bir_kernel = cached
    bass2jax._flametex_neff_cache = True


def _build_nc():
    if "nc" in _NC_CACHE:
        return _NC_CACHE["nc"]
    f32 = mybir.dt.float32
    nc = bacc.Bacc("TRN2")
    blob = nc.dram_tensor("blob", (KA, AW), f32, kind="ExternalInput")
    out_c = nc.dram_tensor("out_c", (MT, NMT * B), f32, kind="ExternalOutput")
    NT = len(C0_GROUPS)
    starts = [B + sum(C0_GROUPS[:j]) for j in range(NT)]

    with tile.TileContext(nc) as tc:
        with (
            tc.tile_pool(name="ap", bufs=1) as ap,
            tc.tile_pool(name="op", bufs=1) as op,
            tc.tile_pool(name="pp", bufs=1, space="PSUM") as pp,
        ):
            a = ap.tile([KC, 2 * AW], f32, tag="a")
            a3 = a[:, :].rearrange("p (c w) -> p c w", c=2)

            g0w = B + C0_GROUPS[0]
            nc.sync.dma_start(a3[0:KC, 0, 0:g0w], blob[0:KC, 0:g0w])
            for j in range(1, NT):
                lo = starts[j]
                nc.sync.dma_start(
                    a3[0:KC, 0, lo : lo + C0_GROUPS[j]],
                    blob[0:KC, lo : lo + C0_GROUPS[j]],
                )
            step = AW // N_C1
            cuts = [0] + [step * i for i in range(1, N_C1)] + [AW]
            for i in range(N_C1):
                nc.gpsimd.dma_start(
                    a3[0:KC1, 1, cuts[i] : cuts[i + 1]],
                    blob[KC:KA, cuts[i] : cuts[i + 1]],
                )

            hps = pp.tile([B, 512], f32, tag="hold")
            for _ in range(N_HOLD):
                nc.tensor.matmul(
                    hps[:, 0:8], a3[:, 0, 0:B], a3[:, 0, B : B + 8],
                    start=True, stop=True,
                )

            # one open accumulation group at a time: HW PSUM group state is
            # bank-granular, so the c0/c1 pair for each m-tile must close
            # before the next tile's pair opens
            ps = pp.tile([MT, NMT * B], f32, tag="ps")
            lo = B
            for mt, mh in enumerate(MT_HEIGHTS):
                nc.tensor.matmul(
                    ps[0:mh, mt * B : (mt + 1) * B],
                    a3[:, 0, lo : lo + mh],
                    a3[:, 0, 0:B],
                    start=True,
                    stop=False,
                )
                nc.tensor.matmul(
                    ps[0:mh, mt * B : (mt + 1) * B],
                    a3[0:KC1, 1, lo : lo + mh],
                    a3[0:KC1, 1, 0:B],
                    start=False,
                    stop=True,
                )
                lo += mh

            ot = op.tile([MT, NMT * B], f32, tag="ot")
            nc.vector.tensor_copy(ot[:, :], ps[:, :])
            nc.sync.dma_start(out_c[:, :], ot[:, :])

    nc.finalize()
    _NC_CACHE["nc"] = nc
    return nc


def kernel(texcode, uv_coords, texture_mean, texture_basis):
    texcode = np.asarray(texcode, dtype=np.float32)
    uv = np.asarray(uv_coords, dtype=np.float32)
    mean = np.asarray(texture_mean, dtype=np.float32).reshape(V)
    basis = np.asarray(texture_basis, dtype=np.float32).reshape(V, K)

    # replicate reference index math exactly in float32
    x = np.clip((uv[:, 0] * np.float32(256.0)).astype(np.int32), 0, 255)
    y = np.clip(
        ((np.float32(1.0) - uv[:, 1]) * np.float32(256.0)).astype(np.int32), 0, 255
    )
    # flat index into the (786432,) texture for output row r = n*3 + c:
    #   v = (2y)*512*3 + (2x)*3 + (2 - c)
    base = 3072 * y.astype(np.int64) + 6 * x.astype(np.int64)
    vidx = (base[:, None] + np.array([2, 1, 0], dtype=np.int64)[None, :]).reshape(-1)

    at = np.zeros((KA, ROWS_PAD), dtype=np.float32)
    at[:K, :ROWS] = basis[vidx].T
    at[K, :ROWS] = mean[vidx]
    xt = np.empty((KA, B), dtype=np.float32)
    xt[:K, :] = texcode.T
    xt[K, :] = 1.0

    _install_neff_cache()
    nc = _build_nc()
    in_maps = []
    for i in range(N_CORES):
        blob = np.empty((KA, AW), dtype=np.float32)
        blob[:, :B] = xt
        blob[:, B:] = at[:, i * PER_CORE : (i + 1) * PER_CORE]
        in_maps.append({"blob": blob})
    res = run_bass_kernel_spmd(nc, in_maps, core_ids=list(range(N_CORES)))

    # out_c[core][p, mt*8 + b] = R[core*1888 + sum(heights[:mt]) + p, b]
    r_parts = []
    for r in res.results:
        arr = r["out_c"].reshape(MT, NMT, B).transpose(1, 0, 2)  # (tile, row, b)
        r_parts.append(
            np.concatenate(
                [arr[:-1].reshape((NMT - 1) * MT, B), arr[-1, : MT_HEIGHTS[-1]]]
            )
        )
    r_full = np.concatenate(r_parts, axis=0)[:ROWS]  # (15069, 8)
    out = r_full.reshape(N_UV, 3, B).transpose(2, 1, 0)  # (B, 3, N_UV)
    return np.ascontiguousarray(out)



# revision 9
# speedup vs baseline: 1.6232x; 1.6232x over previous
"""FLAMETex kernel for Trainium2 (8 NeuronCores, Bass/Tile).

Reference computes tex = mean + basis @ texcode^T over the FULL 786432-row
texture, then downsamples 2x, flips channels (BGR), and gathers 5023 UV
points.  Only 3*5023 = 15069 texture rows can ever reach the output, and
the row indices depend only on uv_coords (an input).  So: compute the
gather indices on the host, gather the needed basis/mean rows, and run a
small (15360 x 200) @ (200 x 8) GEMM on device, row-sharded over the 8
cores (1920 rows each: 15 m-tiles of 128).

Numerics: the rel-err gate is 2e-2; the mean term (sigma ~1) dominates the
basis term (sigma ~0.14).  The basis GEMM runs in fp8e4m3 with a power-of-2
split of the scales (basis*16, texcode/16 -- products are exact in fp32
PSUM), measured ~4e-3 output rel err.  The mean stays fp32 and is added by
the DVE during the PSUM drain.

Per-core device layout (all DMA descriptors >= 512B so no RMW penalty):
  c0 (128, 1928) fp8: [texcode/16 rows 0:128 | basis*16 rows 0:128]^T
  c1 ( 72, 1928) fp8: contraction rows 128:200 of the same
  mx (128,  128) f32: mean in drain layout [p, mt*8+b], cols 120:128 zero
  out_c (128, 128) f32: R-shard, out_c[p, mt*8+b] = R[mt*128+p, b]

Cost-model-guided structure (TimelineSim 10880 -> ~6.3us/core):
  - c0/c1/mx stream on the sync-engine HWDGE back-to-back (descriptor-gen
    pipelines with the transfers; transfers serialize on the DMA engines);
  - per m-tile ONE fp8 accumulation pair (c0 start / c1 stop) into a single
    (128, 120) PSUM bank -- fp8 matmuls cost 1 cycle/row vs fp32's 4;
  - drain is one DVE tensor_add (PSUM + mean tile -> SBUF), so the mean DMA
    sits OFF the matmul critical path;
  - the output leaves via a PREPARED SWDGE scatter-add: descriptors are
    generated on the Pool engine early (prepare_only=True), and after the
    drain a trigger_dma fires them -- skipping the HWDGE gen (625ns) and
    DGE->DMA delay (650ns) a plain dma_start would pay in the tail.
    run_bass_kernel_spmd pre-zeros ExternalOutput buffers, so '+=' into
    out_c with identity indices (an iota on partitions 0:16) is a plain
    store.
"""

import hashlib
import os
import shutil

import numpy as np
from ml_dtypes import float8_e4m3

import concourse.bacc as bacc
import concourse.bass2jax as bass2jax
import concourse.mybir as mybir
import concourse.tile as tile
from concourse.bass_utils import run_bass_kernel_spmd

B = 8
K = 200
N_UV = 5023
V = 786432
ROWS = 3 * N_UV          # 15069 gathered texture rows
N_CORES = 8
PER_CORE = 1920          # 15 m-tiles of 128; 8 * 1920 = 15360 >= 15069
ROWS_PAD = N_CORES * PER_CORE
KC = 128                 # first contraction chunk (partition dim)
KC1 = K - KC             # 72 rows in the second chunk
AW = B + PER_CORE        # 1928
MT = 128                 # m-tile height (PSUM partitions)
NMT = 15
OC = 128                 # out_c cols: 120 data + 8 pad -> 512B descriptors
SCALE = 16.0             # basis*16 / texcode/16: keeps both in e4m3 normal range

_NC_CACHE = {}
_NEFF_CACHE_ROOT = "/tmp/bass_neff_cache"


def _install_neff_cache():
    """Cache compiled NEFFs by BIR content hash across processes.

    The bass2jax neuronx_cc_hook recompiles the identical BIR (a multi-
    minute walrus run with birsim enabled) on every fresh process. The
    kernel's BIR serialization is deterministic, so a sha256-keyed copy of
    the NEFF makes repeat cold starts ~2s instead of minutes. Falls back
    to the original compile on any cache error.
    """
    if getattr(bass2jax, "_flametex_neff_cache", False):
        return
    orig = getattr(bass2jax, "compile_bir_kernel", None)
    if orig is None:
        return

    def cached(bir_json, tmpdir, neff_name="file.neff"):
        key = hashlib.sha256(bir_json).hexdigest()
        cpath = os.path.join(_NEFF_CACHE_ROOT, key, "file.neff")
        dst = os.path.join(tmpdir, neff_name)
        try:
            if os.path.exists(cpath):
                shutil.copy(cpath, dst)
                return dst
        except OSError:
            pass
        neff = orig(bir_json, tmpdir, neff_name=neff_name)
        try:
            os.makedirs(os.path.dirname(cpath), exist_ok=True)
            tmp = cpath + f".tmp{os.getpid()}"
            shutil.copy(neff, tmp)
            os.replace(tmp, cpath)
        except OSError:
            pass
        return neff

    bass2jax.compile_bir_kernel = cached
    bass2jax._flametex_neff_cache = True


def _retarget_orphan_dmasw_waits(nc):
    """Point Tile's orphan DMASW-lane waits at the prep's descriptor sem.

    For a prepare_only SWDGE DMA, Tile books the data-completion tick on a
    DMASW lane and emits end-of-kernel waits against that lane's semaphore,
    but the actual completion increment is the `sem=` baked into the
    descriptors at prep time (Tile's lane sem does not exist yet when the
    descriptor is generated).  Rewrite any wait on a never-incremented
    DMASW* semaphore to the baked descriptor sem -- same event, and the
    rewrite holds on hardware too (the SDMA final descriptor writes the
    baked sem's INC register).
    """
    fn = nc.m.functions[0]
    updated_ids = set()
    desc_sem = None  # (id, ant_name) of the baked swdge_dma sem
    for bb in fn.blocks:
        for inst in bb.instructions:
            si = inst.sync_info
            if si is None:
                continue
            for u in si.on_update:
                updated_ids.add(u.id)
                nm = getattr(u, "ant_name", None)
                if nm and "swdge_dma" in nm:
                    desc_sem = (u.id, nm)
    assert desc_sem is not None, "no baked swdge_dma sem found"
    for bb in fn.blocks:
        for inst in bb.instructions:
            si = inst.sync_info
            if si is None:
                continue
            for w in si.on_wait:
                if (
                    w.ant_name
                    and w.ant_name.startswith("DMASW")
                    and w.id not in updated_ids
                ):
                    w.id, w.ant_name = desc_sem


def _build_nc():
    if "nc" in _NC_CACHE:
        return _NC_CACHE["nc"]
    f32 = mybir.dt.float32
    fp8 = mybir.dt.float8e4
    i16 = mybir.dt.int16
    nc = bacc.Bacc("TRN2")
    c0 = nc.dram_tensor("c0", (KC, AW), fp8, kind="ExternalInput")
    c1 = nc.dram_tensor("c1", (KC1, AW), fp8, kind="ExternalInput")
    mx = nc.dram_tensor("mx", (MT, OC), f32, kind="ExternalInput")
    out_c = nc.dram_tensor("out_c", (MT, OC), f32, kind="ExternalOutput")

    with tile.TileContext(nc) as tc:
        with (
            tc.tile_pool(name="ap", bufs=1) as ap,
            tc.tile_pool(name="op", bufs=1) as op,
            tc.tile_pool(name="pp", bufs=1, space="PSUM") as pp,
        ):
            a0 = ap.tile([KC, AW], fp8, tag="a0")
            a1 = ap.tile([KC1, AW], fp8, tag="a1")
            amx = ap.tile([MT, OC], f32, tag="amx")
            idx = ap.tile([128, 8], i16, tag="idx")
            pm = ap.tile([128, 1], i16, tag="pm")
            ot = op.tile([MT, OC], f32, tag="ot")
            ps = pp.tile([MT, NMT * B], f32, tag="ps")

            # input streams: biggest first so the DMA engines never idle
            nc.sync.dma_start(a0[:, :], c0[:, :])
            nc.sync.dma_start(a1[:, :], c1[:, :])
            nc.sync.dma_start(amx[:, :], mx[:, :])

            # identity scatter indices in the SWDGE wrap order: idx number n
            # lives at partition n%16, column n//16 (ucode reads idxs
            # column-major through a 16-channel port), and each of the 8 Q7
            # cores reads its own 16-partition window, so the 16-partition
            # block must be replicated across all 8 groups:
            #   idx[p, j] = (p % 16) + 16*j
            # Engine APs must start at partition 0, so build it with
            # full-partition ops: 16j (iota) + per-partition scalar (p & 15).
            nc.gpsimd.iota(idx[:, :], pattern=[[16, 8]], base=0, channel_multiplier=0)
            nc.gpsimd.iota(pm[:, :], pattern=[[1, 1]], base=0, channel_multiplier=1)
            nc.vector.tensor_scalar(
                pm[:, :], pm[:, :], 15, None, op0=mybir.AluOpType.bitwise_and
            )
            # 16j and (p & 15) occupy disjoint bits, so OR == ADD here (the
            # AP-scalar 'add' path requires f32; 'bitwise_or' takes int16).
            # AP-scalar (TensorScalarPtr) only exists on the DVE, not Pool.
            nc.vector.tensor_scalar(
                idx[:, :], idx[:, :], pm[:, :], None, op0=mybir.AluOpType.bitwise_or
            )
            nc.vector.memset(ot[:, NMT * B : OC], 0.0)

            # out_c descriptors prepared early on Pool; data deps (ot) defer
            # to the trigger below
            dma_sem = nc.alloc_semaphore("swdge_dma")
            ot3 = ot[:, :].rearrange("p (c w) -> p c w", c=1)
            nc.gpsimd.dma_scatter_add(
                out_c[:, :], ot3, idx[:, :], OC, OC, OC,
                prepare_only=True, sem=dma_sem,
            )

            # one open accumulation group at a time: HW PSUM group state is
            # bank-granular, so each tile's c0/c1 pair must close before the
            # next tile's pair opens
            for mt_i in range(NMT):
                lo = B + mt_i * MT
                col = mt_i * B
                nc.tensor.matmul(
                    ps[:, col : col + B], a0[:, lo : lo + MT], a0[:, 0:B],
                    start=True, stop=False,
                )
                nc.tensor.matmul(
                    ps[:, col : col + B], a1[:, lo : lo + MT], a1[:, 0:B],
                    start=False, stop=True,
                )

            # drain: out = product + mean in one DVE pass
            nc.vector.tensor_add(ot[:, 0 : NMT * B], ps[:, :], amx[:, 0 : NMT * B])
            nc.gpsimd.trigger_dma(count=None)

    _retarget_orphan_dmasw_waits(nc)
    nc.finalize()
    _NC_CACHE["nc"] = nc
    return nc


def _host_prep(texcode, uv_coords, texture_mean, texture_basis):
    """Gather + quantize on host; returns per-core in_maps."""
    texcode = np.asarray(texcode, dtype=np.float32)
    uv = np.asarray(uv_coords, dtype=np.float32)
    mean = np.asarray(texture_mean, dtype=np.float32).reshape(V)
    basis = np.asarray(texture_basis, dtype=np.float32).reshape(V, K)

    # replicate reference index math exactly in float32
    x = np.clip((uv[:, 0] * np.float32(256.0)).astype(np.int32), 0, 255)
    y = np.clip(
        ((np.float32(1.0) - uv[:, 1]) * np.float32(256.0)).astype(np.int32), 0, 255
    )
    # flat index into the (786432,) texture for output row r = n*3 + c:
    #   v = (2y)*512*3 + (2x)*3 + (2 - c)
    base = 3072 * y.astype(np.int64) + 6 * x.astype(np.int64)
    vidx = (base[:, None] + np.array([2, 1, 0], dtype=np.int64)[None, :]).reshape(-1)

    bt = basis[vidx] * np.float32(SCALE)        # (15069, 200)
    bt8 = np.zeros((K, ROWS_PAD), dtype=float8_e4m3)
    bt8[:, :ROWS] = bt.T.astype(float8_e4m3)
    xt8 = (texcode.T * np.float32(1.0 / SCALE)).astype(float8_e4m3)  # (200, 8)

    mean_pad = np.zeros(ROWS_PAD, dtype=np.float32)
    mean_pad[:ROWS] = mean[vidx]

    in_maps = []
    for i in range(N_CORES):
        sl = slice(i * PER_CORE, (i + 1) * PER_CORE)
        c0 = np.zeros((KC, AW), dtype=float8_e4m3)
        c0[:, :B] = xt8[:KC]
        c0[:, B:] = bt8[:KC, sl]
        c1 = np.zeros((KC1, AW), dtype=float8_e4m3)
        c1[:, :B] = xt8[KC:]
        c1[:, B:] = bt8[KC:, sl]
        mxc = np.zeros((MT, OC), dtype=np.float32)
        mxc[:, : NMT * B] = np.repeat(
            mean_pad[sl].reshape(NMT, MT).T, B, axis=1
        )
        in_maps.append({"c0": c0, "c1": c1, "mx": mxc})
    return in_maps


def _unshard(results):
    # out_c[core][p, mt*8 + b] = R[core*1920 + mt*128 + p, b]
    r_parts = []
    for r in results:
        arr = r["out_c"][:, : NMT * B].reshape(MT, NMT, B).transpose(1, 0, 2)
        r_parts.append(arr.reshape(PER_CORE, B))
    r_full = np.concatenate(r_parts, axis=0)[:ROWS]  # (15069, 8)
    out = r_full.reshape(N_UV, 3, B).transpose(2, 1, 0)  # (B, 3, N_UV)
    return np.ascontiguousarray(out.astype(np.float32))


def kernel(texcode, uv_coords, texture_mean, texture_basis):
    in_maps = _host_prep(texcode, uv_coords, texture_mean, texture_basis)
    _install_neff_cache()
    nc = _build_nc()
    res = run_bass_kernel_spmd(nc, in_maps, core_ids=list(range(N_CORES)))
    return _unshard(res.results)
